# revision 10
# baseline (speedup 1.0000x reference)
"""Trainium2 Bass kernel for nn_CNN_RNN (select-GRU -> compact -> 2xGRU -> KimCNN).

Sharding: pure data-parallel, batch 64 -> 8 cores x 8.
Device NEFF1: select-gate input projection (fp32) + fp16-compensated select GRU scan
  -> per-(t,b) argmax margins.
Host: argmax bits -> stable-compaction gather indices (pure data movement) -> gathered
  embedding fold (bf16).
Device NEFF2: layer input projections (bf16), two masked GRU scans (bf16,
  weight-stationary), Kim-CNN convs as shifted matmuls, max-pool+relu, output linear.
"""
import numpy as np
import ml_dtypes

import concourse.bass as bass
import concourse.mybir as mybir
from concourse import bacc
from concourse.tile import TileContext
from contextlib import ExitStack

F32, F16, BF16 = mybir.dt.float32, mybir.dt.float16, mybir.dt.bfloat16
AF = mybir.ActivationFunctionType
ALU = mybir.AluOpType
PE, DVE, ACT = mybir.EngineType.PE, mybir.EngineType.DVE, mybir.EngineType.Activation

B, T, E, H, NF = 64, 512, 768, 512, 256
FS = (3, 4, 5)
NC = 8
BL = B // NC          # batch per core
UNROLL = 16
NITER = T // UNROLL


# ---------------------------------------------------------------- NEFF1 ----

def build_neff1():
    nc = bacc.Bacc("TRN2", target_bir_lowering=False, debug=False, num_devices=NC)
    embT_in = nc.dram_tensor("embT", [128, 6 * T * BL], F32, kind="ExternalInput").ap()
    WihcT_in = nc.dram_tensor("WihcT", [6, 128, 1536], F32, kind="ExternalInput").ap()
    WTc_in = nc.dram_tensor("WTc", [4, 128, 1536], F16, kind="ExternalInput").ap()
    wdpk_in = nc.dram_tensor("wdpk", [128, 8], F16, kind="ExternalInput").ap()
    biasC_in = nc.dram_tensor("biasC", [128, 12], F32, kind="ExternalInput").ap()
    margins_out = nc.dram_tensor("margins", [T * BL], F32, kind="ExternalOutput").ap()

    with TileContext(nc) as tc, ExitStack() as ctx:
        wpool = ctx.enter_context(tc.tile_pool(name="w", bufs=1))
        dpool = ctx.enter_context(tc.tile_pool(name="dram", bufs=1, space="DRAM"))
        gic = dpool.tile([128, NITER * UNROLL * 96], F32, tag="gic")

        WihcT = []
        for k in range(6):
            wt = wpool.tile([128, 1536], F32, tag=f"wihc{k}")
            nc.sync.dma_start(out=wt, in_=WihcT_in[k])
            WihcT.append(wt)
        WTc = []
        for k in range(4):
            wt = wpool.tile([128, 1536], F16, tag=f"wtc{k}")
            nc.sync.dma_start(out=wt, in_=WTc_in[k])
            WTc.append(wt)
        wdpk = wpool.tile([128, 8], F16, tag="wdpk")
        nc.sync.dma_start(out=wdpk, in_=wdpk_in)
        biasC = wpool.tile([128, 12], F32, tag="biasC")
        nc.sync.dma_start(out=biasC, in_=biasC_in)

        # --- phase A: gi_c = embT^T-free matmul (fp32, exact) + bias, to DRAM
        epool = ctx.enter_context(tc.tile_pool(name="emb", bufs=2))
        stpool = ctx.enter_context(tc.tile_pool(name="stage", bufs=2))
        ppool = ctx.enter_context(tc.tile_pool(name="psA", bufs=2, space="PSUM"))
        for nch in range(8):
            et = epool.tile([128, 6, 512], F32, tag="et")
            nc.sync.dma_start(out=et, in_=embT_in.rearrange("p (k c) -> p k c", k=6)[:, :, nch * 512:(nch + 1) * 512])
            stage = stpool.tile([128, 6144], F32, tag="stageA")
            st4 = stage.rearrange("p (i j c b) -> p i j c b", i=4, j=16, c=12)
            for m in range(12):
                ps = ppool.tile([128, 512], F32, tag="psA")
                for k in range(6):
                    nc.tensor.matmul(ps, WihcT[k][:, m * 128:(m + 1) * 128], et[:, k, :],
                                     start=(k == 0), stop=(k == 5))
                # psum col = t'*8+b, t' = i*16+j  ->  stage[p, i, j, m, b]
                nc.vector.tensor_scalar(
                    out=st4[:, :, :, m, :],
                    in0=ps.rearrange("p (i j b) -> p i j b", i=4, j=16),
                    scalar1=biasC[:, m:m + 1], scalar2=None, op0=ALU.add)
            nc.sync.dma_start(out=gic[:, nch * 6144:(nch + 1) * 6144], in_=stage)

        # --- phase B: select scan (fp16 W, split-fp16 h, fp32 psum)
        spool = ctx.enter_context(tc.tile_pool(name="selstate", bufs=1))
        gpool = ctx.enter_context(tc.tile_pool(name="selgi", bufs=3))
        ppoolB = ctx.enter_context(tc.tile_pool(name="psB", bufs=2, space="PSUM"))
        tpool = ctx.enter_context(tc.tile_pool(name="seltmp", bufs=3))
        mpool = ctx.enter_context(tc.tile_pool(name="selmarg", bufs=3))

        hT = spool.tile([128, 32], F32, tag="selhT")
        hpk = spool.tile([128, 64], F16, tag="selhpk")
        nc.vector.memset(hT, 0.0)
        nc.vector.memset(hpk, 0.0)

        with tc.For_i(0, NITER, 1, hint_engines=(PE, DVE, ACT)) as it:
            gi = gpool.tile([128, UNROLL * 96], F32, tag="selgi")
            nc.sync.dma_start(out=gi, in_=gic[:, bass.ds(it * (UNROLL * 96), UNROLL * 96)])
            marg = mpool.tile([1, UNROLL * 8], F32, tag="selmarg")
            for j in range(UNROLL):
                ps_rz = ppoolB.tile([128, 128], F32, tag="ps_rz")
                ps_n = ppoolB.tile([128, 64], F32, tag="ps_n")
                ps_m = ppoolB.tile([1, 16], F32, tag="ps_m")
                for m in range(12):
                    ps = ps_rz[:, m * 16:(m + 1) * 16] if m < 8 else ps_n[:, (m - 8) * 16:(m - 7) * 16]
                    for k in range(4):
                        nc.tensor.matmul(ps, WTc[k][:, m * 128:(m + 1) * 128],
                                         hpk[:, k * 16:(k + 1) * 16],
                                         start=(k == 0), stop=(k == 3))
                gslice = gi[:, j * 96:(j + 1) * 96]
                rz_hi = ps_rz.rearrange("p (m s) -> p m s", s=16)[:, :, 0:8]
                rz_lo = ps_rz.rearrange("p (m s) -> p m s", s=16)[:, :, 8:16]
                a = tpool.tile([128, 64], F32, tag="a")
                nc.vector.tensor_add(out=a.rearrange("p (m s) -> p m s", s=8), in0=rz_hi,
                                     in1=gslice[:, 0:64].rearrange("p (m s) -> p m s", s=8))
                a2 = tpool.tile([128, 64], F32, tag="a2")
                nc.vector.tensor_add(out=a2.rearrange("p (m s) -> p m s", s=8),
                                     in0=a.rearrange("p (m s) -> p m s", s=8), in1=rz_lo)
                rz = tpool.tile([128, 64], F32, tag="rz")
                nc.scalar.activation(rz, a2, AF.Sigmoid)
                n_hi = ps_n.rearrange("p (m s) -> p m s", s=16)[:, :, 0:8]
                n_lo = ps_n.rearrange("p (m s) -> p m s", s=16)[:, :, 8:16]
                t2a = tpool.tile([128, 32], F32, tag="t2a")
                nc.vector.tensor_mul(out=t2a.rearrange("p (m s) -> p m s", s=8), in0=n_hi,
                                     in1=rz[:, 0:32].rearrange("p (m s) -> p m s", s=8))
                t2b = tpool.tile([128, 32], F32, tag="t2b")
                nc.vector.tensor_mul(out=t2b.rearrange("p (m s) -> p m s", s=8), in0=n_lo,
                                     in1=rz[:, 0:32].rearrange("p (m s) -> p m s", s=8))
                u1 = tpool.tile([128, 32], F32, tag="u1")
                nc.vector.tensor_add(out=u1, in0=t2a, in1=gslice[:, 64:96])
                u = tpool.tile([128, 32], F32, tag="u")
                nc.vector.tensor_add(out=u, in0=u1, in1=t2b)
                nn_ = tpool.tile([128, 32], F32, tag="nn_")
                nc.scalar.activation(nn_, u, AF.Tanh)
                d = tpool.tile([128, 32], F32, tag="d")
                nc.vector.tensor_sub(out=d, in0=hT, in1=nn_)
                e = tpool.tile([128, 32], F32, tag="e")
                nc.vector.tensor_mul(out=e, in0=d, in1=rz[:, 32:64])
                nc.vector.tensor_add(out=hT, in0=e, in1=nn_)
                hpk3 = hpk.rearrange("p (k s) -> p k s", s=16)
                hT3 = hT.rearrange("p (k s) -> p k s", s=8)
                nc.vector.tensor_copy(out=hpk3[:, :, 0:8], in_=hT3)
                nc.vector.tensor_sub(out=hpk3[:, :, 8:16], in0=hT3, in1=hpk3[:, :, 0:8])
                for k in range(4):
                    nc.tensor.matmul(ps_m, wdpk[:, k * 2:k * 2 + 1], hpk[:, k * 16:(k + 1) * 16],
                                     start=(k == 0), stop=False)
                    nc.tensor.matmul(ps_m, wdpk[:, k * 2 + 1:k * 2 + 2], hpk[:, k * 16:(k + 1) * 16],
                                     start=False, stop=(k == 3))
                mc = mpool.tile([1, 8], F32, tag="mc")
                nc.vector.tensor_copy(out=mc, in_=ps_m[:, 0:8])
                nc.vector.tensor_add(out=marg[:, j * 8:(j + 1) * 8], in0=mc, in1=ps_m[:, 8:16])
            nc.sync.dma_start(out=margins_out[bass.ds(it * (UNROLL * 8), UNROLL * 8)], in_=marg)
    nc.compile()
    return nc


# ---------------------------------------------------------------- NEFF2 ----

def emit_layer_scan(nc, tc, ctx, name, WhT, gi_dram, mask, masku, ybuf, ycols, n_it):
    """Masked bf16 GRU scan. WhT: 4x sbuf [128,1536] bf16. gi_dram: [128, NITER*1536] bf16.
    mask: sbuf [128, T*BL] bf16 (1/0). ybuf: sbuf [128, 4*ycols] bf16 out (col c*ycols + t*8+b)."""
    spool = ctx.enter_context(tc.tile_pool(name=f"{name}st", bufs=1))
    gpool = ctx.enter_context(tc.tile_pool(name=f"{name}gi", bufs=3))
    ppool = ctx.enter_context(tc.tile_pool(name=f"{name}ps", bufs=2, space="PSUM"))
    tpool = ctx.enter_context(tc.tile_pool(name=f"{name}tmp", bufs=3))

    h16 = spool.tile([128, 32], BF16, tag=f"{name}h16")
    nc.vector.memset(h16, 0.0)
    yb4 = ybuf.rearrange("p (c q) -> p c q", c=4)

    with tc.For_i(0, n_it, 1, hint_engines=(PE, DVE, ACT)) as it:
        gi = gpool.tile([128, UNROLL * 96], BF16, tag=f"{name}gi")
        nc.sync.dma_start(out=gi, in_=gi_dram[:, bass.ds(it * (UNROLL * 96), UNROLL * 96)])
        for j in range(UNROLL):
            tcol = it * UNROLL * 8 + j * 8
            ps_rz = ppool.tile([128, 64], F32, tag=f"{name}ps_rz")
            ps_n = ppool.tile([128, 32], F32, tag=f"{name}ps_n")
            for m in range(12):
                ps = ps_rz[:, m * 8:(m + 1) * 8] if m < 8 else ps_n[:, (m - 8) * 8:(m - 7) * 8]
                for k in range(4):
                    nc.tensor.matmul(ps, WhT[k][:, m * 128:(m + 1) * 128],
                                     h16[:, k * 8:(k + 1) * 8],
                                     start=(k == 0), stop=(k == 3))
            gslice = gi[:, j * 96:(j + 1) * 96]
            a = tpool.tile([128, 64], F32, tag=f"{name}a")
            nc.vector.tensor_add(out=a, in0=ps_rz, in1=gslice[:, 0:64])
            rz = tpool.tile([128, 64], F32, tag=f"{name}rz")
            nc.scalar.activation(rz, a, AF.Sigmoid)
            t2 = tpool.tile([128, 32], F32, tag=f"{name}t2")
            nc.vector.tensor_mul(out=t2, in0=ps_n, in1=rz[:, 0:32])
            u = tpool.tile([128, 32], F32, tag=f"{name}u")
            nc.vector.tensor_add(out=u, in0=t2, in1=gslice[:, 64:96])
            nn_ = tpool.tile([128, 32], F32, tag=f"{name}nn")
            nc.scalar.activation(nn_, u, AF.Tanh)
            d = tpool.tile([128, 32], F32, tag=f"{name}d")
            nc.vector.tensor_sub(out=d, in0=h16, in1=nn_)
            e = tpool.tile([128, 32], F32, tag=f"{name}e")
            nc.vector.tensor_mul(out=e, in0=d, in1=rz[:, 32:64])
            hn16 = tpool.tile([128, 32], BF16, tag=f"{name}hn16")
            nc.vector.tensor_add(out=hn16, in0=e, in1=nn_)
            mview = mask[:, bass.ds(tcol, 8)].unsqueeze(1).broadcast_to([128, 4, 8])
            muview = masku[:, bass.ds(tcol, 8)].unsqueeze(1).broadcast_to([128, 4, 8])
            hn3 = hn16.rearrange("p (c b) -> p c b", c=4)
            # y = m * h'  (zero where invalid)
            nc.vector.tensor_mul(out=yb4[:, :, bass.ds(tcol, 8)], in0=hn3, in1=mview)
            # h <- m ? h' : h
            nc.vector.copy_predicated(out=h16.rearrange("p (c b) -> p c b", c=4),
                                      mask=muview, data=hn3)


def build_neff2(t_pad):
    nc = bacc.Bacc("TRN2", target_bir_lowering=False, debug=False, num_devices=NC)
    TB = t_pad * BL
    NCH = t_pad // 64
    NIT2 = t_pad // 16
    nembT_in = nc.dram_tensor("nembT", [128, 6 * TB], BF16, kind="ExternalInput").ap()
    mask_in = nc.dram_tensor("maskf", [128, TB], BF16, kind="ExternalInput").ap()
    masku_in = nc.dram_tensor("masku", [128, TB], mybir.dt.uint8, kind="ExternalInput").ap()
    Wih0T_in = nc.dram_tensor("Wih0T", [6, 128, 1536], BF16, kind="ExternalInput").ap()
    WhT0_in = nc.dram_tensor("WhT0", [4, 128, 1536], BF16, kind="ExternalInput").ap()
    Wih1T_in = nc.dram_tensor("Wih1T", [4, 128, 1536], BF16, kind="ExternalInput").ap()
    WhT1_in = nc.dram_tensor("WhT1", [4, 128, 1536], BF16, kind="ExternalInput").ap()
    bias0_in = nc.dram_tensor("bias0", [128, 12], F32, kind="ExternalInput").ap()
    bias1_in = nc.dram_tensor("bias1", [128, 12], F32, kind="ExternalInput").ap()
    Wconv_in = nc.dram_tensor("Wconv", [128, 12 * 4 * 256], BF16, kind="ExternalInput").ap()
    bconv_in = nc.dram_tensor("bconv", [128, 6], F32, kind="ExternalInput").ap()
    WoT_in = nc.dram_tensor("WoT", [128, 6], F32, kind="ExternalInput").ap()
    bo_in = nc.dram_tensor("bo", [1, 1], F32, kind="ExternalInput").ap()
    out_dram = nc.dram_tensor("out", [1, BL], F32, kind="ExternalOutput").ap()

    TPAD = t_pad + 16

    with TileContext(nc) as tc, ExitStack() as ctx:
        wpool = ctx.enter_context(tc.tile_pool(name="w2", bufs=1))
        dpool = ctx.enter_context(tc.tile_pool(name="dram2", bufs=1, space="DRAM"))
        gi0d = dpool.tile([128, NIT2 * 1536], BF16, tag="gi0d")
        gi1d = dpool.tile([128, NIT2 * 1536], BF16, tag="gi1d")

        def load_w(name, src, n, dtype=BF16):
            out = []
            for k in range(n):
                wt = wpool.tile([128, 1536], dtype, tag=f"{name}{k}")
                nc.sync.dma_start(out=wt, in_=src[k])
                out.append(wt)
            return out

        Wih0T = load_w("wih0", Wih0T_in, 6)
        WhT0 = load_w("wh0", WhT0_in, 4)
        Wih1T = load_w("wih1", Wih1T_in, 4)
        WhT1 = load_w("wh1", WhT1_in, 4)
        bias0 = wpool.tile([128, 12], F32, tag="bias0")
        nc.sync.dma_start(out=bias0, in_=bias0_in)
        bias1 = wpool.tile([128, 12], F32, tag="bias1")
        nc.sync.dma_start(out=bias1, in_=bias1_in)
        maskf = wpool.tile([128, TB], BF16, tag="maskf")
        nc.sync.dma_start(out=maskf, in_=mask_in)
        masku = wpool.tile([128, TB], mybir.dt.uint8, tag="masku")
        nc.sync.dma_start(out=masku, in_=masku_in)

        # --- gi0 = Wih0 @ nembT + bias0  (nembT streamed per chunk)
        with tc.tile_pool(name="nemb2", bufs=2) as npool, tc.tile_pool(name="st2", bufs=2) as stpool, tc.tile_pool(name="psg0", bufs=2, space="PSUM") as ppool:
            for nch in range(NCH):
                net = npool.tile([128, 6, 512], BF16, tag="net")
                nc.sync.dma_start(out=net, in_=nembT_in.rearrange("p (k c) -> p k c", k=6)[:, :, nch * 512:(nch + 1) * 512])
                stage = stpool.tile([128, 6144], BF16, tag="stage0")
                st4 = stage.rearrange("p (i j c b) -> p i j c b", i=4, j=16, c=12)
                for m in range(12):
                    ps = ppool.tile([128, 512], F32, tag="ps_gi0")
                    for k in range(6):
                        nc.tensor.matmul(ps, Wih0T[k][:, m * 128:(m + 1) * 128],
                                         net[:, k, :],
                                         start=(k == 0), stop=(k == 5))
                    nc.vector.tensor_scalar(
                        out=st4[:, :, :, m, :],
                        in0=ps.rearrange("p (i j b) -> p i j b", i=4, j=16),
                        scalar1=bias0[:, m:m + 1], scalar2=None, op0=ALU.add)
                nc.sync.dma_start(out=gi0d[:, nch * 6144:(nch + 1) * 6144], in_=stage)

        with tc.tile_pool(name="y0p", bufs=1) as y0pool:
            y0buf = y0pool.tile([128, 4 * TB], BF16, tag="y0buf")
            # --- L0 scan
            with ExitStack() as c0:
                emit_layer_scan(nc, tc, c0, "L0", WhT0, gi0d, maskf, masku, y0buf, TB, NIT2)

            # --- gi1 = Wih1 @ y0 + bias1
            y04 = y0buf.rearrange("p (c q) -> p c q", c=4)
            with tc.tile_pool(name="st3", bufs=2) as stpool, tc.tile_pool(name="psg1", bufs=2, space="PSUM") as ppool:
                for nch in range(NCH):
                    stage = stpool.tile([128, 6144], BF16, tag="stage1")
                    st4 = stage.rearrange("p (i j c b) -> p i j c b", i=4, j=16, c=12)
                    for m in range(12):
                        ps = ppool.tile([128, 512], F32, tag="ps_gi1")
                        for k in range(4):
                            nc.tensor.matmul(ps, Wih1T[k][:, m * 128:(m + 1) * 128],
                                             y04[:, k, nch * 512:(nch + 1) * 512],
                                             start=(k == 0), stop=(k == 3))
                        nc.vector.tensor_scalar(
                            out=st4[:, :, :, m, :],
                            in0=ps.rearrange("p (i j b) -> p i j b", i=4, j=16),
                            scalar1=bias1[:, m:m + 1], scalar2=None, op0=ALU.add)
                    nc.sync.dma_start(out=gi1d[:, nch * 6144:(nch + 1) * 6144], in_=stage)

        # --- L1 scan (padded y buffer for conv reads)
        y1buf = wpool.tile([128, 4 * TPAD * BL], BF16, tag="y1buf")
        nc.vector.memset(y1buf, 0.0)
        with ExitStack() as c1:
            emit_layer_scan(nc, tc, c1, "L1", WhT1, gi1d, maskf, masku, y1buf, TPAD * BL, NIT2)

        # --- convs + maxpool + relu + output linear
        Wconv_t = wpool.tile([128, 12 * 4 * 256], BF16, tag="Wconv")
        nc.sync.dma_start(out=Wconv_t, in_=Wconv_in)
        Wconv = Wconv_t.rearrange("p (d k c) -> p d k c", d=12, k=4)
        bconv = wpool.tile([128, 6], F32, tag="bconv")
        nc.sync.dma_start(out=bconv, in_=bconv_in)
        WoT = wpool.tile([128, 6], F32, tag="WoT")
        nc.sync.dma_start(out=WoT, in_=WoT_in)
        cpool = ctx.enter_context(tc.tile_pool(name="cv", bufs=2))
        ppool = ctx.enter_context(tc.tile_pool(name="pscv", bufs=2, space="PSUM"))
        pooled = wpool.tile([128, 48], F32, tag="pooled")
        y14 = y1buf.rearrange("p (c q) -> p c q", c=4)
        dt_base = {3: 0, 4: 3, 5: 7}
        for fi, fs in enumerate(FS):
            for mt in range(2):
                ci = fi * 2 + mt
                macc = cpool.tile([128, 8], F32, tag="macc")
                nc.vector.memset(macc, -1e30)
                for nch in range(NCH):
                    ps = ppool.tile([128, 512], F32, tag="ps_cv")
                    first = True
                    for dt in range(fs):
                        for k in range(4):
                            nc.tensor.matmul(
                                ps, Wconv[:, dt_base[fs] + dt, k, mt * 128:(mt + 1) * 128],
                                y14[:, k, nch * 512 + dt * 8: nch * 512 + dt * 8 + 512],
                                start=first, stop=(dt == fs - 1 and k == 3))
                            first = False
                    nvalid = 64 if nch < NCH - 1 else 65 - fs
                    cm = cpool.tile([128, 8], F32, tag="cm")
                    nc.vector.tensor_reduce(
                        out=cm, in_=ps.rearrange("p (t b) -> p b t", t=64)[:, :, 0:nvalid],
                        axis=mybir.AxisListType.X, op=ALU.max)
                    nc.vector.tensor_max(out=macc, in0=macc, in1=cm)
                if t_pad < T:
                    # windows beyond t_pad read all-zero y -> conv value exactly 0
                    nc.vector.tensor_scalar_max(out=macc, in0=macc, scalar1=0.0)
                nc.scalar.activation(pooled[:, ci * 8:(ci + 1) * 8], macc, AF.Relu,
                                     bias=bconv[:, ci:ci + 1])
        ps_o_t = ppool.tile([128, 8], F32, tag="ps_o")
        ps_o = ps_o_t[0:1, :]
        for ci in range(6):
            nc.tensor.matmul(ps_o, WoT[:, ci:ci + 1], pooled[:, ci * 8:(ci + 1) * 8],
                             start=(ci == 0), stop=(ci == 5))
        bo_sb = wpool.tile([1, 1], F32, tag="bo_sb")
        nc.sync.dma_start(out=bo_sb, in_=bo_in)
        ov = wpool.tile([1, BL], F32, tag="ov")
        nc.vector.tensor_scalar(out=ov, in0=ps_o, scalar1=bo_sb[0:1, 0:1], scalar2=None, op0=ALU.add)
        nc.sync.dma_start(out=out_dram, in_=ov)
    nc.compile()
    return nc


def _make_runner(nc, n_cores):
    import jax
    from jax.sharding import Mesh, PartitionSpec
    from jax.experimental.shard_map import shard_map
    import concourse.bass2jax as b2j
    b2j.install_neuronx_cc_hook()
    pname = nc.partition_id_tensor.name if nc.partition_id_tensor else None
    in_names, out_names, out_avals, zero_outs = [], [], [], []
    for alloc in nc.m.functions[0].allocations:
        if not isinstance(alloc, mybir.MemoryLocationSet):
            continue
        name = alloc.memorylocations[0].name
        if alloc.kind == "ExternalInput":
            if name != pname:
                in_names.append(name)
        elif alloc.kind == "ExternalOutput":
            out_names.append(name)
            shape = tuple(alloc.tensor_shape)
            dtype = mybir.dt.np(alloc.dtype)
            out_avals.append(jax.core.ShapedArray(shape, dtype))
            zero_outs.append(np.zeros(shape, dtype))
    n_params, n_outs = len(in_names), len(out_avals)
    all_in = list(in_names) + list(out_names) + ([pname] if pname else [])
    donate = tuple(range(n_params, n_params + n_outs))

    def _body(*args):
        operands = list(args)
        if pname is not None:
            operands.append(b2j.partition_id_tensor())
        outs = b2j._bass_exec_p.bind(
            *operands, out_avals=tuple(out_avals), in_names=tuple(all_in),
            out_names=tuple(out_names), lowering_input_output_aliases=(),
            sim_require_finite=True, sim_require_nnan=True, nc=nc)
        return tuple(outs)

    mesh = Mesh(np.asarray(jax.devices()[:n_cores]), ("core",))
    fn = jax.jit(shard_map(_body, mesh=mesh,
                           in_specs=(PartitionSpec("core"),) * (n_params + n_outs),
                           out_specs=(PartitionSpec("core"),) * n_outs, check_rep=False),
                 donate_argnums=donate, keep_unused=True)

    def run(in_maps):
        import jax
        per_core = [[np.asarray(m[name]) for name in in_names] for m in in_maps]
        concat_in = [np.concatenate([per_core[c][i] for c in range(n_cores)], axis=0)
                     for i in range(n_params)]
        zeros = [np.zeros((n_cores * z.shape[0], *z.shape[1:]), z.dtype) for z in zero_outs]
        out_arrs = fn(*concat_in, *zeros)
        jax.block_until_ready(out_arrs)
        return [{name: np.asarray(out_arrs[i]).reshape(n_cores, *out_avals[i].shape)[c]
                 for i, name in enumerate(out_names)} for c in range(n_cores)]
    return run


# ------------------------------------------------------------- host glue ----

_cache = {}


def _get_run1():
    if "r1" not in _cache:
        _cache["r1"] = _make_runner(build_neff1(), NC)
    return _cache["r1"]


def _get_run2(t_pad):
    key = ("r2", t_pad)
    if key not in _cache:
        _cache[key] = _make_runner(build_neff2(t_pad), NC)
    return _cache[key]


def _fold_gates_T(W):
    # W: [1536, K] -> [K/128, 128, 1536] lhsT tiles (W.T folded)
    K = W.shape[1]
    return np.ascontiguousarray(W.T.reshape(K // 128, 128, 1536))


def kernel(**inputs):
    emb = np.asarray(inputs["embedded"], np.float32)
    mask = np.asarray(inputs["mask"])
    lens = mask.sum(axis=1).astype(np.int64)
    f32 = np.float32
    Wih_c, Whh_c = np.asarray(inputs["Wih_c"], f32), np.asarray(inputs["Whh_c"], f32)
    bih_c, bhh_c = np.asarray(inputs["bih_c"], f32), np.asarray(inputs["bhh_c"], f32)
    Ws, bs = np.asarray(inputs["Ws"], f32), np.asarray(inputs["bs"], f32)
    Wih0, Whh0 = np.asarray(inputs["Wih0"], f32), np.asarray(inputs["Whh0"], f32)
    bih0, bhh0 = np.asarray(inputs["bih0"], f32), np.asarray(inputs["bhh0"], f32)
    Wih1, Whh1 = np.asarray(inputs["Wih1"], f32), np.asarray(inputs["Whh1"], f32)
    bih1, bhh1 = np.asarray(inputs["bih1"], f32), np.asarray(inputs["bhh1"], f32)
    Wc = {3: np.asarray(inputs["Wc3"], f32), 4: np.asarray(inputs["Wc4"], f32),
          5: np.asarray(inputs["Wc5"], f32)}
    bc = {3: np.asarray(inputs["bc3"], f32), 4: np.asarray(inputs["bc4"], f32),
          5: np.asarray(inputs["bc5"], f32)}
    Wo, bo = np.asarray(inputs["Wo"], f32), np.asarray(inputs["bo"], f32)

    run1 = _get_run1()

    # ---- NEFF1 host prep
    WihcT = _fold_gates_T(Wih_c)
    WTc = _fold_gates_T(Whh_c).astype(np.float16)
    wd = Ws[1] - Ws[0]
    wd_hi = wd.astype(np.float16).astype(f32)
    wd_lo = (wd - wd_hi).astype(np.float16)
    wdpk = np.zeros((128, 8), np.float16)
    for k in range(4):
        wdpk[:, 2 * k] = wd_hi[k * 128:(k + 1) * 128].astype(np.float16)
        wdpk[:, 2 * k + 1] = wd_lo[k * 128:(k + 1) * 128]
    biasC = np.zeros((128, 12), f32)
    bsum = bih_c + bhh_c
    for m in range(12):
        biasC[:, m] = bsum[m * 128:(m + 1) * 128] if m < 8 else bih_c[m * 128:(m + 1) * 128]
    # NOTE: nonzero bhh_c n-gate bias is not folded into the device r*gh_n product;
    # correct for it by adding r*bhh_n ~ impossible hostside -> assert tiny.
    assert np.abs(bhh_c[1024:]).max() == 0.0, "nonzero bhh_c n-gate bias unsupported"

    in1 = []
    for c in range(NC):
        es = emb[c * BL:(c + 1) * BL]                       # [8, T, E]
        embT = np.ascontiguousarray(
            es.reshape(BL, T, 6, 128).transpose(3, 2, 1, 0)).reshape(128, 6 * T * BL)
        in1.append({"embT": embT, "WihcT": WihcT, "WTc": WTc, "wdpk": wdpk,
                    "biasC": biasC})
    res1 = run1(in1)
    margins = np.concatenate([r["margins"].reshape(T, BL).T[None] for r in res1], 0)
    margins = margins.reshape(NC * BL, T)                   # [B, T] (b-major per core)

    # ---- host compaction (bit logic + gather, zero FLOPs)
    thr = bs[0] - bs[1]
    sel = (margins > thr).astype(np.int64)
    t_idx = np.arange(T)[None, :]
    sel[:, 0] = 1
    sel[np.arange(B), lens - 1] = 1
    sel = np.where(t_idx >= lens[:, None], 0, sel)
    nsel = sel.sum(1)
    order = np.argsort(1 - sel, axis=1, kind="stable")
    valid = t_idx < nsel[:, None]
    t_pad = min(T, max(64, int(-(-int(nsel.max()) // 64) * 64)))
    run2 = _get_run2(t_pad)

    # ---- NEFF2 host prep
    Wih0T = _fold_gates_T(Wih0).astype(ml_dtypes.bfloat16)
    WhT0 = _fold_gates_T(Whh0).astype(ml_dtypes.bfloat16)
    Wih1T = _fold_gates_T(Wih1).astype(ml_dtypes.bfloat16)
    WhT1 = _fold_gates_T(Whh1).astype(ml_dtypes.bfloat16)
    bias0 = np.zeros((128, 12), f32)
    b0sum = bih0 + bhh0
    for m in range(12):
        bias0[:, m] = b0sum[m * 128:(m + 1) * 128] if m < 8 else bih0[m * 128:(m + 1) * 128]
    assert np.abs(bhh0[1024:]).max() == 0.0 and np.abs(bhh1[1024:]).max() == 0.0
    bias1 = np.zeros((128, 12), f32)
    b1sum = bih1 + bhh1
    for m in range(12):
        bias1[:, m] = b1sum[m * 128:(m + 1) * 128] if m < 8 else bih1[m * 128:(m + 1) * 128]
    Wconv = np.zeros((12, 4, 128, 256), f32)
    dt_base = {3: 0, 4: 3, 5: 7}
    for fs in FS:
        Wf = Wc[fs][:, 0]                                   # [NF, fs, H]
        for dt in range(fs):
            for k in range(4):
                Wconv[dt_base[fs] + dt, k] = Wf[:, dt, k * 128:(k + 1) * 128].T
    Wconv = np.ascontiguousarray(Wconv.transpose(2, 0, 1, 3)).reshape(128, -1).astype(ml_dtypes.bfloat16)
    bconv = np.zeros((128, 6), f32)
    WoT = np.zeros((128, 6), f32)
    for fi, fs in enumerate(FS):
        for mt in range(2):
            bconv[:, fi * 2 + mt] = bc[fs][mt * 128:(mt + 1) * 128]
            WoT[:, fi * 2 + mt] = Wo[0, fi * 256 + mt * 128: fi * 256 + (mt + 1) * 128]

    in2 = []
    for c in range(NC):
        bsl = slice(c * BL, (c + 1) * BL)
        new_emb = np.take_along_axis(emb[bsl], order[bsl][:, :, None], axis=1)
        new_emb = (new_emb * valid[bsl][:, :, None])[:, :t_pad]
        nembT = np.ascontiguousarray(
            new_emb.reshape(BL, t_pad, 6, 128).transpose(3, 2, 1, 0)
        ).reshape(128, 6 * t_pad * BL).astype(ml_dtypes.bfloat16)
        vs = valid[bsl][:, :t_pad]
        maskf = np.ascontiguousarray(np.broadcast_to(
            vs.T.reshape(1, t_pad * BL), (128, t_pad * BL))).astype(ml_dtypes.bfloat16)
        masku = np.ascontiguousarray(np.broadcast_to(
            vs.T.reshape(1, t_pad * BL), (128, t_pad * BL))).astype(np.uint8)
        in2.append({"nembT": nembT, "maskf": maskf, "masku": masku, "Wih0T": Wih0T, "WhT0": WhT0,
                    "Wih1T": Wih1T, "WhT1": WhT1, "bias0": bias0, "bias1": bias1,
                    "Wconv": Wconv, "bconv": bconv, "WoT": WoT,
                    "bo": bo.reshape(1, 1)})
    res2 = run2(in2)
    out = np.concatenate([r["out"].reshape(BL) for r in res2], 0)
    return out.astype(np.float32)
